# revision 1
# baseline (speedup 1.0000x reference)
"""BiMamba Trainium2 kernel.

8-core sharding: core = (batch b in {0,1}) x (direction in {fwd, rev}) x
(d_inner half in {0,1}).  Each core runs one Mamba branch over its half of
d_inner (1024 channels) for one batch element, producing a partial
contribution to out = y_fwd + y_rev; the host sums the 4 partials per batch.

Layout on device: channels on partitions, sequence position on the free dim.
  Phase 1: in_proj (PE) -> xi, zs(=silu(z)); causal depthwise conv (DVE/GPS)
           + silu -> xc; x_dbl (PE) -> dt/Bm/Cm; dt_proj (PE) + softplus ->
           delta; du = delta*xc; w2 = (xc*Dp)*zs.  Spills delta/du/zs/w2
           (bf16) and Bm/Cm (bf16) to DRAM.
  Phase 2: per (n, d-tile): a_n = exp(-(n+1)*delta) on ACT; b_n =
           du*B_n^bcast; h_n = tensor_tensor_scan(a_n, b_n); m_n =
           h_n*C_n^bcast; y_ssm = sum_n m_n; yT = y_ssm*zs + w2.  Spills yT.
  Phase 3: out_proj (PE) -> partial output [1024 dmodel, L].

The exp scale -(n+1) relies on A_log = log(arange(1, 17)) broadcast over
channels, which setup_inputs() guarantees; kernel() asserts it.
"""

import sys

for _p in ("/opt/trn_rl_repo",):
    if _p not in sys.path:
        sys.path.insert(0, _p)

import numpy as np

import concourse.bass as bass
import concourse.bacc as bacc
import concourse.mybir as mybir
import concourse.tile as tile

# Model dims (hardcoded per contest contract)
D_MODEL = 1024
D_STATE = 16
D_INNER = 2048
DT_RANK = 64
B, L = 2, 2048
DH = D_INNER // 2          # 1024 channels per core
NDT = DH // 128            # 8 d-tiles per core
NKT = D_MODEL // 128       # 8 k-tiles for in_proj contraction

F32 = mybir.dt.float32
F32R = mybir.dt.float32r
BF16 = mybir.dt.bfloat16
ALU = mybir.AluOpType
ACTF = mybir.ActivationFunctionType

LC = 512                   # phase-1 L-chunk
NLC = L // LC

LAST_EXEC_NS = None


def _silu(nc, tc, pool, out_ap, in_ap, bias, native):
    """out = silu(in + bias). native=True uses the HW Silu LUT; otherwise
    composes sigmoid+mul (CoreSim lacks Silu)."""
    if native:
        nc.scalar.activation(out_ap, in_ap, ACTF.Silu, bias=bias, scale=1.0)
    else:
        v = pool.tile([out_ap.shape[0], out_ap.shape[1]], F32, name="silv", tag="silv", bufs=1)
        nc.vector.tensor_scalar(v[:], in_ap, bias if not isinstance(bias, float)
                                else float(bias), None, op0=ALU.add)
        s = pool.tile([out_ap.shape[0], out_ap.shape[1]], F32, name="sils", tag="sils", bufs=1)
        nc.scalar.activation(s[:], v[:], ACTF.Sigmoid)
        nc.vector.tensor_tensor(out_ap, v[:], s[:], op=ALU.mult)


def build_program(native_silu=True):
    nc = bacc.Bacc("TRN2", target_bir_lowering=False, debug=False,
                   num_devices=8)

    xT = nc.dram_tensor("xT", [D_MODEL, L], F32R, kind="ExternalInput")
    w_in = nc.dram_tensor("w_in", [D_MODEL, 2 * DH], F32R, kind="ExternalInput")
    w_xp = nc.dram_tensor("w_xp", [DH, 96], F32R, kind="ExternalInput")
    w_dtp = nc.dram_tensor("w_dtp", [DT_RANK, DH], F32R, kind="ExternalInput")
    w_out = nc.dram_tensor("w_out", [DH, D_MODEL], F32R, kind="ExternalInput")
    # per-channel params: conv_w[0:4], conv_b[4], dtp_b[5], Dp[6]
    chp = nc.dram_tensor("chp", [DH, 7], F32, kind="ExternalInput")
    outp_a = nc.dram_tensor("outp_a", [D_MODEL, L], F32, kind="ExternalOutput")
    outp_b = nc.dram_tensor("outp_b", [D_MODEL, L], F32, kind="ExternalOutput")

    sp_delta = nc.dram_tensor("sp_delta", [DH, L], BF16)
    sp_du = nc.dram_tensor("sp_du", [DH, L], BF16)
    sp_zs = nc.dram_tensor("sp_zs", [DH, L], BF16)
    sp_w2 = nc.dram_tensor("sp_w2", [DH, L], BF16)
    sp_bc = nc.dram_tensor("sp_bc", [32, L], BF16)

    with tile.TileContext(nc) as tc:
        _phase1(nc, tc, xT, w_in, w_xp, w_dtp, chp,
                sp_delta, sp_du, sp_zs, sp_w2, sp_bc, native_silu)
        _phase2(nc, tc, sp_delta, sp_du, sp_zs, sp_w2, sp_bc, w_out,
                outp_a, outp_b)
    nc.finalize()
    return nc


def _phase1(nc, tc, xT, w_in, w_xp, w_dtp, chp,
            sp_delta, sp_du, sp_zs, sp_w2, sp_bc, native_silu):
    with (
        tc.tile_pool(name="p1_win", bufs=1) as win_pool,
        tc.tile_pool(name="p1_wsmall", bufs=1) as wsm_pool,
        tc.tile_pool(name="p1_xt", bufs=1) as xt_pool,
        tc.tile_pool(name="p1_xi", bufs=2) as xi_pool,
        tc.tile_pool(name="p1_xc", bufs=2) as xc_pool,
        tc.tile_pool(name="p1_misc", bufs=2) as misc_pool,
        tc.tile_pool(name="p1_psum", bufs=2, space="PSUM") as psum_pool,
        tc.tile_pool(name="p1_psum96", bufs=2, space="PSUM") as psum96_pool,
    ):
        win_sb = []
        for kt in range(NKT):
            t = win_pool.tile([128, 2 * DH], F32R, name=f"win{kt}", tag=f"win{kt}")
            nc.sync.dma_start(t[:], w_in[kt * 128:(kt + 1) * 128, :])
            win_sb.append(t)
        wxp_sb = wsm_pool.tile([128, NKT * 96], F32R, name="wxp", tag="wxp")
        nc.sync.dma_start(
            wxp_sb[:].rearrange("p (a l) -> p a l", a=NKT),
            w_xp[:].rearrange("(a p) l -> p a l", p=128))
        wdtp_sb = wsm_pool.tile([DT_RANK, DH], F32R, name="wdtp", tag="wdtp")
        nc.sync.dma_start(wdtp_sb[:], w_dtp[:])
        chp_sb = []
        for dt in range(NDT):
            t = wsm_pool.tile([128, 7], F32, name=f"chp{dt}", tag=f"chp{dt}")
            nc.sync.dma_start(t[:], chp[dt * 128:(dt + 1) * 128, :])
            chp_sb.append(t)

        bc_bf = misc_pool.tile([32, L], BF16, name="bc_bf", tag="bc_bf", bufs=1)

        hist = [None] * NDT

        for c in range(NLC):
            lo = c * LC
            xt_sb = xt_pool.tile([128, NKT * LC], F32R, name="xt", tag="xt")
            nc.sync.dma_start(
                xt_sb[:].rearrange("p (a l) -> p a l", a=NKT),
                xT[:, lo:lo + LC].rearrange("(a p) l -> p a l", p=128))

            zs_big = misc_pool.tile([128, NDT * LC], BF16, name="zsbig", tag="zsbig", bufs=1)
            w2_big = misc_pool.tile([128, NDT * LC], BF16, name="w2big", tag="w2big", bufs=1)
            de_big = misc_pool.tile([128, NDT * LC], BF16, name="debig", tag="debig", bufs=1)
            du_big = misc_pool.tile([128, NDT * LC], BF16, name="dubig", tag="dubig", bufs=1)
            xc_list = []
            xcr_list = []
            for dt in range(NDT):
                # in_proj: xi rows
                ps = psum_pool.tile([128, LC], F32, name="ps_xi", tag="ps_xi")
                for kt in range(NKT):
                    nc.tensor.matmul(
                        ps[:],
                        lhsT=win_sb[kt][:, dt * 128:(dt + 1) * 128],
                        rhs=xt_sb[:, kt * LC:(kt + 1) * LC],
                        start=(kt == 0), stop=(kt == NKT - 1))
                xi_new = xi_pool.tile([128, LC + 3], F32, name="xi", tag="xi", bufs=3)
                if c == 0:
                    nc.vector.memset(xi_new[:, 0:3], 0.0)
                else:
                    nc.vector.tensor_copy(xi_new[:, 0:3], hist[dt][:])
                nc.scalar.copy(xi_new[:, 3:LC + 3], ps[:])
                if c < NLC - 1:
                    h_t = xi_pool.tile([128, 3], F32, name="hist", tag=f"hist{dt}", bufs=2)
                    nc.vector.tensor_copy(h_t[:], xi_new[:, LC:LC + 3])
                    hist[dt] = h_t

                # conv (4 causal taps) + bias + silu
                xc_t = xc_pool.tile([128, LC], F32, name="xc", tag=f"xc{dt}")
                wcol = chp_sb[dt]
                nc.vector.tensor_scalar(xc_t[:], xi_new[:, 0:LC],
                                        wcol[:, 0:1], None, op0=ALU.mult)
                nc.vector.scalar_tensor_tensor(
                    out=xc_t[:], in0=xi_new[:, 1:LC + 1], scalar=wcol[:, 1:2],
                    in1=xc_t[:], op0=ALU.mult, op1=ALU.add)
                cvt = misc_pool.tile([128, LC], F32, name="cvt", tag="cvt", bufs=1)
                nc.gpsimd.tensor_scalar(cvt[:], xi_new[:, 2:LC + 2],
                                        wcol[:, 2:3], None, op0=ALU.mult)
                nc.gpsimd.tensor_tensor(xc_t[:], xc_t[:], cvt[:], op=ALU.add)
                nc.vector.scalar_tensor_tensor(
                    out=xc_t[:], in0=xi_new[:, 3:LC + 3], scalar=wcol[:, 3:4],
                    in1=xc_t[:], op0=ALU.mult, op1=ALU.add)
                _silu(nc, tc, misc_pool, xc_t[:], xc_t[:], wcol[:, 4:5], native_silu)
                xcr_t = xc_pool.tile([128, LC], F32R, name="xcr", tag=f"xcr{dt}", bufs=1)
                nc.gpsimd.tensor_copy(xcr_t[:], xc_t[:])
                xc_list.append(xc_t)
                xcr_list.append(xcr_t)

                # in_proj: z rows -> silu -> zs; w2 = (xc*Dp)*zs
                ps2 = psum_pool.tile([128, LC], F32, name="ps_z", tag="ps_z")
                for kt in range(NKT):
                    nc.tensor.matmul(
                        ps2[:],
                        lhsT=win_sb[kt][:, DH + dt * 128:DH + (dt + 1) * 128],
                        rhs=xt_sb[:, kt * LC:(kt + 1) * LC],
                        start=(kt == 0), stop=(kt == NKT - 1))
                zs_t = misc_pool.tile([128, LC], F32, name="zs", tag="zs")
                _silu(nc, tc, misc_pool, zs_t[:], ps2[:], 0.0, native_silu)
                nc.vector.tensor_copy(zs_big[:, dt * LC:(dt + 1) * LC], zs_t[:])
                w2f = misc_pool.tile([128, LC], F32, name="w2f", tag="w2f", bufs=1)
                nc.gpsimd.tensor_scalar(w2f[:], xc_t[:], wcol[:, 6:7], None,
                                        op0=ALU.mult)
                nc.gpsimd.tensor_tensor(w2_big[:, dt * LC:(dt + 1) * LC], w2f[:],
                                        zs_t[:], op=ALU.mult)

            # x_dbl = xp_w @ xc : [96, LC]
            ps96 = psum96_pool.tile([96, LC], F32, name="ps96", tag="ps96")
            for kt in range(NKT):
                nc.tensor.matmul(
                    ps96[:],
                    lhsT=wxp_sb[:, kt * 96:(kt + 1) * 96],
                    rhs=xcr_list[kt][:],
                    start=(kt == 0), stop=(kt == NKT - 1))
            nc.scalar.copy(bc_bf[:, lo:lo + LC], ps96[64:96, :])
            dt_sb = misc_pool.tile([64, LC], F32R, name="dt_sb", tag="dt", bufs=1)
            nc.scalar.copy(dt_sb[:], ps96[0:64, :])

            # delta = softplus(dtp @ dt + dtp_b) = ln(1 + exp(pre))
            for dt in range(NDT):
                psd = psum_pool.tile([128, LC], F32, name="ps_d", tag="ps_d")
                nc.tensor.matmul(
                    psd[:],
                    lhsT=wdtp_sb[:, dt * 128:(dt + 1) * 128],
                    rhs=dt_sb[:],
                    start=True, stop=True)
                u_t = misc_pool.tile([128, LC], F32, name="u_t", tag="u_t", bufs=1)
                nc.scalar.activation(u_t[:], psd[:], ACTF.Exp,
                                     bias=chp_sb[dt][:, 5:6], scale=1.0)
                delta_t = misc_pool.tile([128, LC], F32, name="delta", tag="delta")
                nc.scalar.activation(delta_t[:], u_t[:], ACTF.Ln, bias=1.0, scale=1.0)
                nc.vector.tensor_copy(de_big[:, dt * LC:(dt + 1) * LC], delta_t[:])
                nc.vector.tensor_tensor(du_big[:, dt * LC:(dt + 1) * LC],
                                        delta_t[:], xc_list[dt][:], op=ALU.mult)

            for t_big, sp in ((zs_big, sp_zs), (w2_big, sp_w2),
                              (de_big, sp_delta), (du_big, sp_du)):
                nc.sync.dma_start(
                    sp[:, lo:lo + LC].rearrange("(a p) l -> p a l", p=128),
                    t_big[:].rearrange("p (a l) -> p a l", a=NDT))

        nc.sync.dma_start(sp_bc[:], bc_bf[:])


def _phase2(nc, tc, sp_delta, sp_du, sp_zs, sp_w2, sp_bc, w_out, outp_a, outp_b):
    NSR = 2                 # super-rounds over d-tiles
    DPS = NDT // NSR        # 4 d-tiles per super-round
    NG = 4                  # n-group size
    LH = L // 2             # broadcast tiles come in L-halves
    with (
        tc.tile_pool(name="p2_loads", bufs=1) as load_pool,
        tc.tile_pool(name="p2_bc", bufs=1) as bc_pool,
        tc.tile_pool(name="p2_a", bufs=2) as a_pool,
        tc.tile_pool(name="p2_b", bufs=3) as b_pool,
        tc.tile_pool(name="p2_h", bufs=4) as h_pool,
        tc.tile_pool(name="p2_pair", bufs=1) as pair_pool,
        tc.tile_pool(name="p2_y", bufs=1) as y_pool,
        tc.tile_pool(name="p2_tail", bufs=1) as tail_pool,
        tc.tile_pool(name="p2_psum", bufs=4, space="PSUM") as psum_pool,
    ):
        for sr in range(NSR):
            dts = [sr * DPS + i for i in range(DPS)]
            d0 = dts[0] * 128
            de_l = load_pool.tile([128, DPS * L], BF16, name="de_l", tag="de_l")
            nc.sync.dma_start(
                de_l[:].rearrange("p (a l) -> p a l", a=DPS),
                sp_delta[d0:d0 + DPS * 128, :].rearrange("(a p) l -> p a l", p=128))
            du_l = load_pool.tile([128, DPS * L], BF16, name="du_l", tag="du_l")
            nc.sync.dma_start(
                du_l[:].rearrange("p (a l) -> p a l", a=DPS),
                sp_du[d0:d0 + DPS * 128, :].rearrange("(a p) l -> p a l", p=128))
            wo_l = load_pool.tile([128, DPS * D_MODEL], F32R, name="wo_l",
                                  tag="wo_l")
            nc.sync.dma_start(
                wo_l[:].rearrange("p (a l) -> p a l", a=DPS),
                w_out[d0:d0 + DPS * 128, :].rearrange("(a p) l -> p a l", p=128))
            delta_t = {dt: de_l[:, (dt - dts[0]) * L:(dt - dts[0] + 1) * L]
                       for dt in dts}
            du_t = {dt: du_l[:, (dt - dts[0]) * L:(dt - dts[0] + 1) * L]
                    for dt in dts}
            ysum = {dt: y_pool.tile([128, L], F32, name=f"ys{dt}",
                                    tag=f"ys{dt - dts[0]}")
                    for dt in dts}

            for ng in range(D_STATE // NG):
                ns = [ng * NG + i for i in range(NG)]
                Bb, Cb = {}, {}
                for lh in range(2):
                    Bg = bc_pool.tile([128, NG * LH], BF16, name=f"Bg{lh}",
                                      tag=f"Bg{lh}")
                    nc.sync.dma_start(
                        Bg[:].rearrange("p (a l) -> p a l", a=NG),
                        sp_bc[ns[0]:ns[0] + NG,
                              lh * LH:(lh + 1) * LH].partition_broadcast(128))
                    Cg = bc_pool.tile([128, NG * LH], BF16, name=f"Cg{lh}",
                                      tag=f"Cg{lh}")
                    nc.sync.dma_start(
                        Cg[:].rearrange("p (a l) -> p a l", a=NG),
                        sp_bc[16 + ns[0]:16 + ns[0] + NG,
                              lh * LH:(lh + 1) * LH].partition_broadcast(128))
                    for n in ns:
                        r = n - ns[0]
                        Bb[(n, lh)] = Bg[:, r * LH:(r + 1) * LH]
                        Cb[(n, lh)] = Cg[:, r * LH:(r + 1) * LH]
                for dt in dts:
                    ms = []
                    for n in ns:
                        a_t = a_pool.tile([128, L], F32, name=f"a{n}", tag="a")
                        nc.scalar.activation(a_t[:], delta_t[dt], ACTF.Exp,
                                             scale=-float(n + 1))
                        b_t = b_pool.tile([128, L], BF16, name=f"b{n}", tag="b")
                        b_eng = nc.gpsimd if (n % 4) >= 1 else nc.vector
                        for lh in range(2):
                            b_eng.tensor_tensor(
                                b_t[:, lh * LH:(lh + 1) * LH],
                                du_t[dt][:, lh * LH:(lh + 1) * LH],
                                Bb[(n, lh)], op=ALU.mult)
                        h_t = h_pool.tile([128, L], BF16, name=f"h{n}", tag="h")
                        nc.vector.tensor_tensor_scan(
                            h_t[:], a_t[:], b_t[:], 0.0,
                            op0=ALU.mult, op1=ALU.add)
                        m_eng = nc.gpsimd if (n % 2 == 1) else nc.vector
                        for lh in range(2):
                            m_eng.tensor_tensor(h_t[:, lh * LH:(lh + 1) * LH],
                                                h_t[:, lh * LH:(lh + 1) * LH],
                                                Cb[(n, lh)], op=ALU.mult)
                        ms.append(h_t)
                    p0 = pair_pool.tile([128, L], BF16, name="p0", tag="p0")
                    nc.gpsimd.tensor_tensor(p0[:], ms[0][:], ms[1][:], op=ALU.add)
                    p1 = pair_pool.tile([128, L], BF16, name="p1", tag="p1")
                    nc.gpsimd.tensor_tensor(p1[:], ms[2][:], ms[3][:], op=ALU.add)
                    if ng == 0:
                        nc.gpsimd.tensor_tensor(ysum[dt][:], p0[:], p1[:], op=ALU.add)
                    else:
                        nc.gpsimd.tensor_tensor(p0[:], p0[:], p1[:], op=ALU.add)
                        nc.vector.scalar_tensor_tensor(
                            out=ysum[dt][:], in0=p0[:], scalar=1.0,
                            in1=ysum[dt][:], op0=ALU.mult, op1=ALU.add)

            # tail: yT = ysum*zs + w2 (in place), round to f32r
            yTr = {}
            for dt in dts:
                pd0 = dt * 128
                zs_l = tail_pool.tile([128, L], BF16, name="zs_l", tag="zs_l")
                nc.sync.dma_start(zs_l[:], sp_zs[pd0:pd0 + 128, :])
                w2_l = tail_pool.tile([128, L], BF16, name="w2_l", tag="w2_l")
                nc.sync.dma_start(w2_l[:], sp_w2[pd0:pd0 + 128, :])
                nc.gpsimd.tensor_tensor(ysum[dt][:], ysum[dt][:], zs_l[:],
                                        op=ALU.mult)
                nc.vector.scalar_tensor_tensor(
                    out=ysum[dt][:], in0=w2_l[:], scalar=1.0,
                    in1=ysum[dt][:], op0=ALU.mult, op1=ALU.add)
                yr = tail_pool.tile([128, L], F32R, name=f"yr{dt}",
                                    tag=f"yr{dt - dts[0]}")
                nc.gpsimd.tensor_copy(yr[:], ysum[dt][:])
                yTr[dt] = yr

            # out_proj partial for this super-round
            outp_x = outp_a if sr == 0 else outp_b
            for mt in range(8):
                o_t = y_pool.tile([128, L], F32, name="o_t", tag=f"ys{mt % 2}")
                for c in range(NLC):
                    ps = psum_pool.tile([128, LC], F32, name="ps_o", tag="ps_o")
                    for r, dt in enumerate(dts):
                        nc.tensor.matmul(
                            ps[:],
                            lhsT=wo_l[:, r * D_MODEL + mt * 128:
                                      r * D_MODEL + (mt + 1) * 128],
                            rhs=yTr[dt][:, c * LC:(c + 1) * LC],
                            start=(r == 0), stop=(r == DPS - 1))
                    nc.scalar.copy(o_t[:, c * LC:(c + 1) * LC], ps[:])
                nc.sync.dma_start(outp_x[mt * 128:(mt + 1) * 128, :], o_t[:])


def make_in_maps(inputs):
    x = np.asarray(inputs["x"], np.float32)
    names = ["in_w", "conv_w", "conv_b", "xp_w", "dtp_w", "dtp_b",
             "A_log", "Dvec", "out_w"]
    params = {d: [np.asarray(inputs[k + str(d + 1)], np.float32) for k in names]
              for d in range(2)}
    # the device program hardcodes A_n = -(n+1); verify
    expA = np.log(np.arange(1, D_STATE + 1, dtype=np.float32))
    for d in range(2):
        A_log = params[d][6]
        assert np.allclose(A_log, np.broadcast_to(expA, A_log.shape), atol=1e-6), \
            "A_log does not match the expected log(arange(1,17)) pattern"

    in_maps, metas = [], []
    for core in range(8):
        b = core & 1
        dire = (core >> 1) & 1
        half = (core >> 2) & 1
        in_w, conv_w, conv_b, xp_w, dtp_w, dtp_b, A_log, Dp, out_w = params[dire]
        sl = slice(half * DH, (half + 1) * DH)
        xb = x[b] if dire == 0 else x[b, ::-1]
        chp = np.concatenate([
            conv_w[sl, 0, :],
            conv_b[sl, None],
            dtp_b[sl, None],
            Dp[sl, None],
        ], axis=1).astype(np.float32)
        in_maps.append({
            "xT": np.ascontiguousarray(xb.T),
            "w_in": np.ascontiguousarray(
                np.concatenate([in_w[sl], in_w[D_INNER + half * DH:
                                               D_INNER + (half + 1) * DH]]).T),
            "w_xp": np.ascontiguousarray(xp_w[:, sl].T),
            "w_dtp": np.ascontiguousarray(dtp_w[sl].T),
            "w_out": np.ascontiguousarray(out_w[:, sl].T),
            "chp": np.ascontiguousarray(chp),
        })
        metas.append(b)
    return in_maps, metas


_PROGRAM_CACHE = {}


def kernel(**inputs):
    global LAST_EXEC_NS
    import os
    from concourse.bass_utils import run_bass_kernel_spmd

    if "nc" not in _PROGRAM_CACHE:
        _PROGRAM_CACHE["nc"] = build_program(native_silu=True)
    nc = _PROGRAM_CACHE["nc"]

    in_maps, metas = make_in_maps(inputs)
    trace = os.environ.get("BIMAMBA_TRACE", "0") == "1"
    res = run_bass_kernel_spmd(nc, in_maps, list(range(8)), trace=trace)
    LAST_EXEC_NS = res.exec_time_ns
    out = np.zeros((B, L, D_MODEL), np.float32)
    for core in range(8):
        out[metas[core]] += res.results[core]["outp_a"].T
        out[metas[core]] += res.results[core]["outp_b"].T
    return out



# revision 27
# speedup vs baseline: 1.1293x; 1.1293x over previous
"""BiMamba Trainium2 kernel (v2).

8-core sharding: core = (batch b) x (direction) x (d_inner half).  Each core
runs one Mamba branch over DH=1024 channels for one batch element; host sums
the 4 partials per batch element.

Key structure (vs v1 baseline):
  - PE does all contraction-like work: in_proj, depthwise conv (4 diagonal
    matmuls), x_dbl, dt_proj, the Dvec term (diagonal matmul into PSUM), the
    sum over the 16 SSM states (identity matmuls accumulating in PSUM f32),
    and out_proj.
  - ACT does the silu/softplus activations and most exp(-(n+1)*delta) tiles,
    batched so only ~3 activation-table loads happen.
  - DVE does the 128 sequential scans (the hard floor) plus a tuned share of
    the b = du*B and m = h*C broadcasts; Pool does the rest.
  - Everything is bf16 except the a-tiles (f32) and PSUM accumulation (f32).
    Outputs are written bf16 and summed in f32 on the host.
  - Phase-1 products (xc, zs, delta, du) spill to DRAM and are re-loaded
    transiently per (dt, n-group) so the B/C broadcast tiles (8 states
    resident at a time) fit in SBUF.

The exp scale -(n+1) relies on A_log = log(arange(1, 17)) broadcast over
channels, which setup_inputs() guarantees; kernel() asserts it.
"""

import sys

for _p in ("/opt/trn_rl_repo",):
    if _p not in sys.path:
        sys.path.insert(0, _p)

import numpy as np

import concourse.bass as bass
import concourse.bacc as bacc
import concourse.mybir as mybir
import concourse.tile as tile

# Model dims (hardcoded per contest contract)
D_MODEL = 1024
D_STATE = 16
D_INNER = 2048
DT_RANK = 64
B, L = 2, 2048
DH = D_INNER // 2          # 1024 channels per core
NDT = DH // 128            # 8 d-tiles per core
NKT = D_MODEL // 128       # 8 k-tiles for in_proj contraction

F32 = mybir.dt.float32
BF16 = mybir.dt.bfloat16
ALU = mybir.AluOpType
ACTF = mybir.ActivationFunctionType

LC = 512                   # phase-1 L-chunk (psum bank width in f32)
NLC = L // LC
NG = 8                     # n-group size in phase 2 (B/C tiles resident)

LAST_EXEC_NS = None


def _bm_engine(nc, idx, g):
    """Engine for the b/m broadcast multiplies: DVE share tuned per group."""
    return nc.vector if (idx % 7) < 1 else nc.gpsimd


def build_program():
    nc = bacc.Bacc("TRN2", target_bir_lowering=False, debug=False,
                   num_devices=8)

    xT = nc.dram_tensor("xT", [D_MODEL, L], BF16, kind="ExternalInput")
    w_in = nc.dram_tensor("w_in", [D_MODEL, 2 * DH], BF16, kind="ExternalInput")
    w_xp = nc.dram_tensor("w_xp", [DH, 96], BF16, kind="ExternalInput")
    w_dtp = nc.dram_tensor("w_dtp", [DT_RANK, DH], BF16, kind="ExternalInput")
    w_out = nc.dram_tensor("w_out", [DH, D_MODEL], BF16, kind="ExternalInput")
    ident = nc.dram_tensor("ident", [128, 128], BF16, kind="ExternalInput")
    # per-channel params per dt: conv taps 0-3, conv_b, dtp_b, Dvec
    chp = nc.dram_tensor("chp", [128, NDT * 7], F32, kind="ExternalInput")
    outp_a = nc.dram_tensor("outp_a", [D_MODEL, L], BF16, kind="ExternalOutput")
    outp_b = nc.dram_tensor("outp_b", [D_MODEL, L], BF16, kind="ExternalOutput")

    sp_bc = nc.dram_tensor("sp_bc", [32, L], BF16)
    sp_xc = nc.dram_tensor("sp_xc", [DH, L], BF16)
    sp_de = nc.dram_tensor("sp_de", [DH, L], BF16)
    sp_du = nc.dram_tensor("sp_du", [DH, L], BF16)

    with tile.TileContext(nc) as tc:
        with (
            tc.tile_pool(name="const", bufs=1) as const_pool,
        ):
            ident_sb = const_pool.tile([128, 128], BF16, name="ident",
                                       tag="ident")
            nc.sync.dma_start(ident_sb[:], ident[:])
            chp_sb = const_pool.tile([128, NDT * 7], F32, name="chp", tag="chp")
            nc.sync.dma_start(chp_sb[:], chp[:])

            _phase1(nc, tc, xT, w_in, w_xp, w_dtp, chp_sb,
                    sp_bc, sp_xc, sp_de, sp_du)
            _phase2(nc, tc, sp_bc, sp_xc, sp_de, sp_du, xT, w_in, w_out,
                    chp_sb, ident_sb, outp_a, outp_b)
    nc.finalize()
    return nc


def _phase1(nc, tc, xT, w_in, w_xp, w_dtp, chp_sb,
            sp_bc, sp_xc, sp_de, sp_du):
    # in_proj + conv(Pool) + silu; x_dbl / dt_proj / softplus / du
    # interleaved per chunk so ACT and Pool trail the PE in_proj stream.
    with (
        tc.tile_pool(name="a_big", bufs=1) as big_pool,      # xc, zs, de, du
        tc.tile_pool(name="a_small", bufs=1) as small_pool,  # dt_sb, bc_sb
        tc.tile_pool(name="a_win", bufs=1) as win_pool,
        tc.tile_pool(name="a_xt", bufs=2) as xt_pool,
        tc.tile_pool(name="a_xi", bufs=2) as xi_pool,
        tc.tile_pool(name="a_u", bufs=2) as u_pool,
        tc.tile_pool(name="a_ps", bufs=4, space="PSUM") as ps_pool,
        tc.tile_pool(name="a_ps96", bufs=2, space="PSUM") as ps96_pool,
    ):
        xc_sb = [big_pool.tile([128, L], BF16, name=f"xc{dt}", tag=f"xc{dt}")
                 for dt in range(NDT)]
        de_sb = [big_pool.tile([128, L], BF16, name=f"de{dt}", tag=f"de{dt}")
                 for dt in range(NDT)]
        du_sb = [big_pool.tile([128, L], BF16, name=f"du{dt}", tag=f"du{dt}")
                 for dt in range(NDT)]
        dt_sb = small_pool.tile([DT_RANK, L], BF16, name="dt_sb", tag="dt_sb")
        bc_sb = small_pool.tile([32, L], BF16, name="bc_sb", tag="bc_sb")

        win_sb = []
        for kt in range(NKT):
            t = win_pool.tile([128, 2 * DH], BF16, name=f"win{kt}",
                              tag=f"win{kt}")
            nc.sync.dma_start(t[:], w_in[kt * 128:(kt + 1) * 128, :])
            win_sb.append(t)
        wxp_sb = win_pool.tile([128, NKT * 96], BF16, name="wxp", tag="wxp")
        nc.sync.dma_start(
            wxp_sb[:].rearrange("p (a l) -> p a l", a=NKT),
            w_xp[:].rearrange("(a p) l -> p a l", p=128))
        wdtp_sb = win_pool.tile([DT_RANK, DH], BF16, name="wdtp", tag="wdtp")
        nc.sync.dma_start(wdtp_sb[:], w_dtp[:])

        hist = [None] * NDT
        for c in range(NLC):
            lo = c * LC
            xt_sb = xt_pool.tile([128, NKT * LC], BF16, name="xt", tag="xt")
            nc.sync.dma_start(
                xt_sb[:].rearrange("p (a l) -> p a l", a=NKT),
                xT[:, lo:lo + LC].rearrange("(a p) l -> p a l", p=128))

            for dt in range(NDT):
                # in_proj xi rows
                ps = ps_pool.tile([128, LC], F32, name="ps_xi", tag="ps_xi")
                for kt in range(NKT):
                    nc.tensor.matmul(
                        ps[:],
                        lhsT=win_sb[kt][:, dt * 128:(dt + 1) * 128],
                        rhs=xt_sb[:, kt * LC:(kt + 1) * LC],
                        start=(kt == 0), stop=(kt == NKT - 1))
                xi = xi_pool.tile([128, LC + 3], BF16, name="xi", tag="xi",
                                  bufs=3)
                if c == 0:
                    nc.vector.memset(xi[:, 0:3], 0.0)
                else:
                    nc.vector.tensor_copy(xi[:, 0:3], hist[dt][:])
                nc.vector.tensor_copy(xi[:, 3:LC + 3], ps[:])
                if c < NLC - 1:
                    h_t = xi_pool.tile([128, 3], BF16, name="hist",
                                       tag=f"hist{dt}", bufs=2)
                    nc.vector.tensor_copy(h_t[:], xi[:, LC:LC + 3])
                    hist[dt] = h_t

                # conv: 4 causal taps on Pool (f32 accumulate)
                cv = xi_pool.tile([128, LC], F32, name="cv", tag="cv", bufs=2)
                nc.gpsimd.tensor_scalar(cv[:], xi[:, 0:LC],
                                        chp_sb[:, 7 * dt:7 * dt + 1], None,
                                        op0=ALU.mult)
                nc.vector.scalar_tensor_tensor(
                    out=cv[:], in0=xi[:, 1:1 + LC],
                    scalar=chp_sb[:, 7 * dt + 1:7 * dt + 2],
                    in1=cv[:], op0=ALU.mult, op1=ALU.add)
                cv2 = xi_pool.tile([128, LC], F32, name="cv2", tag="cv2",
                                   bufs=2)
                nc.gpsimd.tensor_scalar(cv2[:], xi[:, 2:2 + LC],
                                        chp_sb[:, 7 * dt + 2:7 * dt + 3],
                                        None, op0=ALU.mult)
                nc.gpsimd.tensor_tensor(cv[:], cv[:], cv2[:], op=ALU.add)
                nc.vector.scalar_tensor_tensor(
                    out=cv[:], in0=xi[:, 3:3 + LC],
                    scalar=chp_sb[:, 7 * dt + 3:7 * dt + 4],
                    in1=cv[:], op0=ALU.mult, op1=ALU.add)
                nc.scalar.activation(xc_sb[dt][:, lo:lo + LC], cv[:],
                                     ACTF.Silu,
                                     bias=chp_sb[:, 7 * dt + 4:7 * dt + 5],
                                     scale=1.0)

            # x_dbl for this chunk
            ps96 = ps96_pool.tile([96, LC], F32, name="ps96", tag="ps96")
            for kt in range(NKT):
                nc.tensor.matmul(
                    ps96[:],
                    lhsT=wxp_sb[:, kt * 96:(kt + 1) * 96],
                    rhs=xc_sb[kt][:, lo:lo + LC],
                    start=(kt == 0), stop=(kt == NKT - 1))
            nc.scalar.copy(dt_sb[:, lo:lo + LC], ps96[0:64, :])
            nc.scalar.copy(bc_sb[:, lo:lo + LC], ps96[64:96, :])

            # spill xc/bc as chunks complete
            nc.sync.dma_start(sp_bc[:, lo:lo + LC], bc_sb[:, lo:lo + LC])
            for dt in range(NDT):
                nc.sync.dma_start(sp_xc[dt * 128:(dt + 1) * 128, lo:lo + LC],
                                  xc_sb[dt][:, lo:lo + LC])

        # dt_proj + softplus + du, batched (one activation-table switch)
        for c in range(NLC):
            lo = c * LC
            for dt in range(NDT):
                psd = ps_pool.tile([128, LC], F32, name="psd", tag="psd",
                                   bufs=2)
                nc.tensor.matmul(
                    psd[:],
                    lhsT=wdtp_sb[:, dt * 128:(dt + 1) * 128],
                    rhs=dt_sb[:, lo:lo + LC],
                    start=True, stop=True)
                u_t = u_pool.tile([128, LC], BF16, name="u_t", tag="u_t")
                nc.scalar.activation(u_t[:], psd[:], ACTF.Exp,
                                     bias=chp_sb[:, 7 * dt + 5:7 * dt + 6],
                                     scale=1.0)
                nc.scalar.activation(de_sb[dt][:, lo:lo + LC], u_t[:],
                                     ACTF.Ln, bias=1.0, scale=1.0)
                nc.gpsimd.tensor_tensor(du_sb[dt][:, lo:lo + LC],
                                        de_sb[dt][:, lo:lo + LC],
                                        xc_sb[dt][:, lo:lo + LC],
                                        op=ALU.mult)
                dsl = slice(dt * 128, (dt + 1) * 128)
                nc.sync.dma_start(sp_de[dsl, lo:lo + LC],
                                  de_sb[dt][:, lo:lo + LC])
                nc.sync.dma_start(sp_du[dsl, lo:lo + LC],
                                  du_sb[dt][:, lo:lo + LC])


def _phase2(nc, tc, sp_bc, sp_xc, sp_de, sp_du, xT, w_in, w_out,
            chp_sb, ident_sb, outp_a, outp_b):
    GN = 4                       # states per group
    NGR = D_STATE // GN          # 4 groups
    with (
        tc.tile_pool(name="b_wout", bufs=1) as wout_pool,
        tc.tile_pool(name="b_bc", bufs=1) as bc_pool,
        tc.tile_pool(name="b_yt", bufs=1) as yt_pool,
        tc.tile_pool(name="b_xtg", bufs=1) as xtg_pool,
        tc.tile_pool(name="b_in", bufs=2) as in_pool,     # de/du/xc loads
        tc.tile_pool(name="b_a", bufs=2) as a_pool,
        tc.tile_pool(name="b_bh", bufs=2) as bh_pool,
        tc.tile_pool(name="b_tail", bufs=2) as tail_pool,
        tc.tile_pool(name="b_o", bufs=1) as o_pool,
        tc.tile_pool(name="b_ps", bufs=2, space="PSUM") as psy_pool,
    ):
        yT = [yt_pool.tile([128, L], BF16, name=f"yT{dt}", tag=f"yT{dt}")
              for dt in range(NDT)]

        bm_idx = 0
        for g in range(NGR):
            n0 = g * GN
            Bg = bc_pool.tile([128, GN * L], BF16, name=f"Bg{g}", tag="Bg")
            nc.sync.dma_start(
                Bg[:].rearrange("p (a l) -> p a l", a=GN),
                sp_bc[n0:n0 + GN, :].partition_broadcast(128))
            Cg = bc_pool.tile([128, GN * L], BF16, name=f"Cg{g}", tag="Cg")
            nc.sync.dma_start(
                Cg[:].rearrange("p (a l) -> p a l", a=GN),
                sp_bc[16 + n0:16 + n0 + GN, :].partition_broadcast(128))

            if g == NGR - 1:
                # z-path weights + xT for the deferred in_proj z recompute
                winz_sb = []
                for kt in range(NKT):
                    t = xtg_pool.tile([128, DH], BF16, name=f"wz{kt}",
                                      tag=f"wz{kt}")
                    nc.sync.dma_start(t[:],
                                      w_in[kt * 128:(kt + 1) * 128, DH:])
                    winz_sb.append(t)
                xtg = []
                for c in range(NLC):
                    t = xtg_pool.tile([128, NKT * LC], BF16, name=f"xtg{c}",
                                      tag=f"xtg{c}")
                    nc.sync.dma_start(
                        t[:].rearrange("p (a l) -> p a l", a=NKT),
                        xT[:, c * LC:(c + 1) * LC].rearrange(
                            "(a p) l -> p a l", p=128))
                    xtg.append(t)

            for dt in range(NDT):
                de_t = in_pool.tile([128, L], BF16, name="de_t", tag="de_t")
                nc.sync.dma_start(de_t[:], sp_de[dt * 128:(dt + 1) * 128, :])
                du_t = in_pool.tile([128, L], BF16, name="du_t", tag="du_t")
                nc.sync.dma_start(du_t[:], sp_du[dt * 128:(dt + 1) * 128, :])
                if g == 0:
                    xc_t = in_pool.tile([128, L], BF16, name="xc_t", tag="xc_t")
                    nc.sync.dma_start(xc_t[:],
                                      sp_xc[dt * 128:(dt + 1) * 128, :])
                ps_y = [psy_pool.tile([128, LC], F32, name=f"psy{q}",
                                      tag=f"psy{q}") for q in range(NLC)]

                for ni in range(GN):
                    n = n0 + ni
                    a_t = a_pool.tile([128, L], F32, name=f"a{n}", tag="a")
                    nc.scalar.activation(a_t[:], de_t[:], ACTF.Exp,
                                         scale=-float(n + 1))

                    b_t = bh_pool.tile([128, L], BF16, name=f"b{n}", tag="b")
                    _bm_engine(nc, bm_idx, g).tensor_tensor(
                        b_t[:], du_t[:], Bg[:, ni * L:(ni + 1) * L],
                        op=ALU.mult)
                    bm_idx += 1

                    h_t = bh_pool.tile([128, L], BF16, name=f"h{n}", tag="h")
                    nc.vector.tensor_tensor_scan(
                        h_t[:], a_t[:], b_t[:], 0.0,
                        op0=ALU.mult, op1=ALU.add)

                    m_t = bh_pool.tile([128, L], BF16, name=f"m{n}", tag="m")
                    _bm_engine(nc, bm_idx, g).tensor_tensor(
                        m_t[:], h_t[:], Cg[:, ni * L:(ni + 1) * L],
                        op=ALU.mult)
                    bm_idx += 1

                    for q in range(NLC):
                        nc.tensor.matmul(
                            ps_y[q][:],
                            lhsT=ident_sb[:],
                            rhs=m_t[:, q * LC:(q + 1) * LC],
                            start=(ni == 0),
                            stop=(ni == GN - 1))

                if g == NGR - 1:
                    # zs' = 2*silu(z) = z*(1+tanh(z/2)); the 1/2 is folded
                    # into w_out on the host.  tanh shares the exp table set,
                    # so g3 runs with zero activation-table switches.
                    zs_t = in_pool.tile([128, L], BF16, name="zs_t", tag="zs_t")
                    for c in range(NLC):
                        ps_z = psy_pool.tile([128, LC], F32, name="ps_z",
                                             tag=f"psy{c}")
                        for kt in range(NKT):
                            nc.tensor.matmul(
                                ps_z[:],
                                lhsT=winz_sb[kt][:, dt * 128:(dt + 1) * 128],
                                rhs=xtg[c][:, kt * LC:(kt + 1) * LC],
                                start=(kt == 0), stop=(kt == NKT - 1))
                        th_t = tail_pool.tile([128, LC], BF16, name="th",
                                              tag="th")
                        nc.scalar.activation(th_t[:], ps_z[:], ACTF.Tanh,
                                             scale=0.5)
                        tz_t = tail_pool.tile([128, LC], BF16, name="tz",
                                              tag="tz")
                        nc.vector.tensor_tensor(tz_t[:], ps_z[:], th_t[:],
                                                op=ALU.mult)
                        nc.vector.scalar_tensor_tensor(
                            out=zs_t[:, c * LC:(c + 1) * LC], in0=tz_t[:],
                            scalar=1.0, in1=ps_z[:], op0=ALU.mult,
                            op1=ALU.add)

                # tail: fold this group's psum into yT
                for q in range(NLC):
                    lo = q * LC
                    if g == 0:
                        # yT = xc*Dvec + psum  (gating applied in last group)
                        nc.vector.scalar_tensor_tensor(
                            out=yT[dt][:, lo:lo + LC],
                            in0=xc_t[:, lo:lo + LC],
                            scalar=chp_sb[:, 7 * dt + 6:7 * dt + 7],
                            in1=ps_y[q][:], op0=ALU.mult, op1=ALU.add)
                    elif g < NGR - 1:
                        nc.vector.tensor_tensor(yT[dt][:, lo:lo + LC],
                                                yT[dt][:, lo:lo + LC],
                                                ps_y[q][:], op=ALU.add)
                    else:
                        t_q = tail_pool.tile([128, LC], F32, name="t_q",
                                             tag="t_q")
                        nc.vector.tensor_tensor(t_q[:], ps_y[q][:],
                                                yT[dt][:, lo:lo + LC],
                                                op=ALU.add)
                        nc.gpsimd.tensor_tensor(yT[dt][:, lo:lo + LC],
                                                t_q[:],
                                                zs_t[:, lo:lo + LC],
                                                op=ALU.mult)

                if g == NGR - 1 and dt == 4:
                    _out_proj_half(nc, tc, psy_pool, o_pool, wout_pool,
                                   w_out, yT, 0, outp_a, nc.vector)
        _out_proj_half(nc, tc, psy_pool, o_pool, wout_pool, w_out, yT,
                       1, outp_b, None)


def _out_proj_half(nc, tc, psy_pool, o_pool, wout_pool, w_out, yT, half, outp,
                   cp_eng):
    dts = [half * 4 + i for i in range(4)]
    wout_sb = {}
    for dt in dts:
        t = wout_pool.tile([128, D_MODEL], BF16, name=f"wo{dt}",
                           tag=f"wo{dt % 4}")
        nc.sync.dma_start(t[:], w_out[dt * 128:(dt + 1) * 128, :])
        wout_sb[dt] = t
    for mt in range(8):
        o_t = o_pool.tile([128, L], BF16, name="o_t", tag="o_t")
        for c in range(NLC):
            ps = psy_pool.tile([128, LC], F32, name="ps_o", tag=f"psy{c}")
            for r, dt in enumerate(dts):
                nc.tensor.matmul(
                    ps[:],
                    lhsT=wout_sb[dt][:, mt * 128:(mt + 1) * 128],
                    rhs=yT[dt][:, c * LC:(c + 1) * LC],
                    start=(r == 0), stop=(r == 3))
            if cp_eng is None:
                nc.scalar.copy(o_t[:, c * LC:(c + 1) * LC], ps[:])
            else:
                cp_eng.tensor_copy(o_t[:, c * LC:(c + 1) * LC], ps[:])
        nc.sync.dma_start(outp[mt * 128:(mt + 1) * 128, :], o_t[:])


def make_in_maps(inputs):
    import ml_dtypes
    bf16 = ml_dtypes.bfloat16
    x = np.asarray(inputs["x"], np.float32)
    names = ["in_w", "conv_w", "conv_b", "xp_w", "dtp_w", "dtp_b",
             "A_log", "Dvec", "out_w"]
    params = {d: [np.asarray(inputs[k + str(d + 1)], np.float32) for k in names]
              for d in range(2)}
    # the device program hardcodes A_n = -(n+1); verify
    expA = np.log(np.arange(1, D_STATE + 1, dtype=np.float32))
    for d in range(2):
        A_log = params[d][6]
        assert np.allclose(A_log, np.broadcast_to(expA, A_log.shape),
                           atol=1e-6), \
            "A_log does not match the expected log(arange(1,17)) pattern"

    ident = np.eye(128, dtype=np.float32)

    in_maps, metas = [], []
    for core in range(8):
        b = core & 1
        dire = (core >> 1) & 1
        half = (core >> 2) & 1
        in_w, conv_w, conv_b, xp_w, dtp_w, dtp_b, A_log, Dp, out_w = params[dire]
        sl = slice(half * DH, (half + 1) * DH)
        xb = x[b] if dire == 0 else x[b, ::-1]
        chpm = np.zeros((128, NDT * 7), np.float32)
        for dt in range(NDT):
            ch = slice(half * DH + dt * 128, half * DH + (dt + 1) * 128)
            for k in range(4):
                chpm[:, 7 * dt + k] = conv_w[ch, 0, k]
            chpm[:, 7 * dt + 4] = conv_b[ch]
            chpm[:, 7 * dt + 5] = dtp_b[ch]
            chpm[:, 7 * dt + 6] = Dp[ch]
        in_maps.append({
            "xT": np.ascontiguousarray(xb.T).astype(bf16),
            "w_in": np.ascontiguousarray(
                np.concatenate([in_w[sl], in_w[D_INNER + half * DH:
                                               D_INNER + (half + 1) * DH]]).T
            ).astype(bf16),
            "w_xp": np.ascontiguousarray(xp_w[:, sl].T).astype(bf16),
            "w_dtp": np.ascontiguousarray(dtp_w[sl].T).astype(bf16),
            "w_out": np.ascontiguousarray(out_w[:, sl].T * 0.5).astype(bf16),
            "ident": ident.astype(bf16),
            "chp": chpm.astype(np.float32),
        })
        metas.append(b)
    return in_maps, metas


_PROGRAM_CACHE = {}


def kernel(**inputs):
    global LAST_EXEC_NS
    import os
    from concourse.bass_utils import run_bass_kernel_spmd

    if "nc" not in _PROGRAM_CACHE:
        _PROGRAM_CACHE["nc"] = build_program()
    nc = _PROGRAM_CACHE["nc"]

    in_maps, metas = make_in_maps(inputs)
    trace = os.environ.get("BIMAMBA_TRACE", "0") == "1"
    res = run_bass_kernel_spmd(nc, in_maps, list(range(8)), trace=trace)
    LAST_EXEC_NS = res.exec_time_ns
    out = np.zeros((B, L, D_MODEL), np.float32)
    for core in range(8):
        out[metas[core]] += np.asarray(res.results[core]["outp_a"],
                                       np.float32).T
        out[metas[core]] += np.asarray(res.results[core]["outp_b"],
                                       np.float32).T
    return out


# revision 36
# speedup vs baseline: 1.2445x; 1.1020x over previous
"""BiMamba Trainium2 kernel (v2).

8-core sharding: core = (batch b) x (direction) x (d_inner half).  Each core
runs one Mamba branch over DH=1024 channels for one batch element; host sums
the 4 partials per batch element.

Key structure (vs v1 baseline):
  - PE does all contraction-like work: in_proj, depthwise conv (4 diagonal
    matmuls), x_dbl, dt_proj, the Dvec term (diagonal matmul into PSUM), the
    sum over the 16 SSM states (identity matmuls accumulating in PSUM f32),
    and out_proj.
  - ACT does the silu/softplus activations and most exp(-(n+1)*delta) tiles,
    batched so only ~3 activation-table loads happen.
  - DVE does the 128 sequential scans (the hard floor) plus a tuned share of
    the b = du*B and m = h*C broadcasts; Pool does the rest.
  - Everything is bf16 except the a-tiles (f32) and PSUM accumulation (f32).
    Outputs are written bf16 and summed in f32 on the host.
  - Phase-1 products (xc, zs, delta, du) spill to DRAM and are re-loaded
    transiently per (dt, n-group) so the B/C broadcast tiles (8 states
    resident at a time) fit in SBUF.

The exp scale -(n+1) relies on A_log = log(arange(1, 17)) broadcast over
channels, which setup_inputs() guarantees; kernel() asserts it.
"""

import sys

for _p in ("/opt/trn_rl_repo",):
    if _p not in sys.path:
        sys.path.insert(0, _p)

import numpy as np

import concourse.bass as bass
import concourse.bacc as bacc
import concourse.mybir as mybir
import concourse.tile as tile

# Model dims (hardcoded per contest contract)
D_MODEL = 1024
D_STATE = 16
D_INNER = 2048
DT_RANK = 64
B, L = 2, 2048
DH = D_INNER // 2          # 1024 channels per core
NDT = DH // 128            # 8 d-tiles per core
NKT = D_MODEL // 128       # 8 k-tiles for in_proj contraction

F32 = mybir.dt.float32
BF16 = mybir.dt.bfloat16
ALU = mybir.AluOpType
ACTF = mybir.ActivationFunctionType

LC = 512                   # phase-1 L-chunk (psum bank width in f32)
NLC = L // LC
NG = 8                     # n-group size in phase 2 (B/C tiles resident)

LAST_EXEC_NS = None


def _bm_engine(nc, idx, g):
    """Engine for the b/m broadcast multiplies: DVE share tuned per group."""
    return nc.vector if (idx % 5) < 1 else nc.gpsimd


def build_program():
    nc = bacc.Bacc("TRN2", target_bir_lowering=False, debug=False,
                   num_devices=8)

    xT = nc.dram_tensor("xT", [D_MODEL, L], BF16, kind="ExternalInput")
    w_in = nc.dram_tensor("w_in", [D_MODEL, 2 * DH], BF16, kind="ExternalInput")
    w_xp = nc.dram_tensor("w_xp", [DH, 96], BF16, kind="ExternalInput")
    w_dtp = nc.dram_tensor("w_dtp", [DT_RANK, DH], BF16, kind="ExternalInput")
    w_out = nc.dram_tensor("w_out", [DH, D_MODEL], BF16, kind="ExternalInput")
    ident = nc.dram_tensor("ident", [128, 128], BF16, kind="ExternalInput")
    # per-channel params per dt: conv taps 0-3, conv_b, dtp_b, Dvec
    chp = nc.dram_tensor("chp", [128, NDT * 7], F32, kind="ExternalInput")
    outp_a = nc.dram_tensor("outp_a", [D_MODEL, L], BF16, kind="ExternalOutput")
    outp_b = nc.dram_tensor("outp_b", [D_MODEL, L], BF16, kind="ExternalOutput")

    sp_bc = nc.dram_tensor("sp_bc", [32, L], BF16)
    sp_xc = nc.dram_tensor("sp_xc", [DH, L], BF16)
    sp_zs = nc.dram_tensor("sp_zs", [DH, L], BF16)
    sp_de = nc.dram_tensor("sp_de", [DH, L], BF16)

    with tile.TileContext(nc) as tc:
        with (
            tc.tile_pool(name="const", bufs=1) as const_pool,
        ):
            ident_sb = const_pool.tile([128, 128], BF16, name="ident",
                                       tag="ident")
            nc.sync.dma_start(ident_sb[:], ident[:])
            chp_sb = const_pool.tile([128, NDT * 7], F32, name="chp", tag="chp")
            nc.sync.dma_start(chp_sb[:], chp[:])
            dt_sb = const_pool.tile([DT_RANK, L], BF16, name="dt_sb",
                                    tag="dt_sb")
            wdtp_sb = const_pool.tile([DT_RANK, DH], BF16, name="wdtp",
                                      tag="wdtp")
            nc.sync.dma_start(wdtp_sb[:], w_dtp[:])

            _phase1(nc, tc, xT, w_in, w_xp, chp_sb, dt_sb,
                    sp_bc, sp_xc, sp_zs)
            _phase2(nc, tc, sp_bc, sp_xc, sp_zs, sp_de, w_out,
                    chp_sb, ident_sb, dt_sb, wdtp_sb, outp_a, outp_b)
    nc.finalize()
    return nc


def _phase1(nc, tc, xT, w_in, w_xp, chp_sb, dt_sb,
            sp_bc, sp_xc, sp_zs):
    # in_proj + conv(Pool) + silu; x_dbl / dt_proj / softplus / du
    # interleaved per chunk so ACT and Pool trail the PE in_proj stream.
    with (
        tc.tile_pool(name="a_big", bufs=1) as big_pool,      # xc, zs, de, du
        tc.tile_pool(name="a_small", bufs=1) as small_pool,  # dt_sb, bc_sb
        tc.tile_pool(name="a_win", bufs=1) as win_pool,
        tc.tile_pool(name="a_xt", bufs=2) as xt_pool,
        tc.tile_pool(name="a_xi", bufs=2) as xi_pool,
        tc.tile_pool(name="a_u", bufs=2) as u_pool,
        tc.tile_pool(name="a_ps", bufs=2, space="PSUM") as ps_pool,
        tc.tile_pool(name="a_ps96", bufs=2, space="PSUM") as ps96_pool,
    ):
        xc_sb = [big_pool.tile([128, L], BF16, name=f"xc{dt}", tag=f"xc{dt}")
                 for dt in range(NDT)]
        bc_sb = small_pool.tile([32, L], BF16, name="bc_sb", tag="bc_sb")

        win_sb = []
        for kt in range(NKT):
            t = win_pool.tile([128, 2 * DH], BF16, name=f"win{kt}",
                              tag=f"win{kt}")
            nc.sync.dma_start(t[:], w_in[kt * 128:(kt + 1) * 128, :])
            win_sb.append(t)
        wxp_sb = win_pool.tile([128, NKT * 96], BF16, name="wxp", tag="wxp")
        nc.sync.dma_start(
            wxp_sb[:].rearrange("p (a l) -> p a l", a=NKT),
            w_xp[:].rearrange("(a p) l -> p a l", p=128))
        hist = [None] * NDT
        for c in range(NLC):
            lo = c * LC
            xt_sb = xt_pool.tile([128, NKT * LC], BF16, name="xt", tag="xt")
            nc.sync.dma_start(
                xt_sb[:].rearrange("p (a l) -> p a l", a=NKT),
                xT[:, lo:lo + LC].rearrange("(a p) l -> p a l", p=128))

            for dt in range(NDT):
                # in_proj xi rows
                ps = ps_pool.tile([128, LC], F32, name="ps_xi", tag="ps_xi")
                for kt in range(NKT):
                    nc.tensor.matmul(
                        ps[:],
                        lhsT=win_sb[kt][:, dt * 128:(dt + 1) * 128],
                        rhs=xt_sb[:, kt * LC:(kt + 1) * LC],
                        start=(kt == 0), stop=(kt == NKT - 1))
                xi = xi_pool.tile([128, LC + 3], BF16, name="xi", tag="xi",
                                  bufs=3)
                if c == 0:
                    nc.vector.memset(xi[:, 0:3], 0.0)
                else:
                    nc.vector.tensor_copy(xi[:, 0:3], hist[dt][:])
                nc.vector.tensor_copy(xi[:, 3:LC + 3], ps[:])
                if c < NLC - 1:
                    h_t = xi_pool.tile([128, 3], BF16, name="hist",
                                       tag=f"hist{dt}", bufs=2)
                    nc.vector.tensor_copy(h_t[:], xi[:, LC:LC + 3])
                    hist[dt] = h_t

                # conv: 4 causal taps on Pool (f32 accumulate)
                cv = xi_pool.tile([128, LC], F32, name="cv", tag="cv", bufs=2)
                nc.gpsimd.tensor_scalar(cv[:], xi[:, 0:LC],
                                        chp_sb[:, 7 * dt:7 * dt + 1], None,
                                        op0=ALU.mult)
                nc.vector.scalar_tensor_tensor(
                    out=cv[:], in0=xi[:, 1:1 + LC],
                    scalar=chp_sb[:, 7 * dt + 1:7 * dt + 2],
                    in1=cv[:], op0=ALU.mult, op1=ALU.add)
                cv2 = xi_pool.tile([128, LC], F32, name="cv2", tag="cv2",
                                   bufs=2)
                nc.gpsimd.tensor_scalar(cv2[:], xi[:, 2:2 + LC],
                                        chp_sb[:, 7 * dt + 2:7 * dt + 3],
                                        None, op0=ALU.mult)
                nc.gpsimd.tensor_tensor(cv[:], cv[:], cv2[:], op=ALU.add)
                nc.vector.scalar_tensor_tensor(
                    out=cv[:], in0=xi[:, 3:3 + LC],
                    scalar=chp_sb[:, 7 * dt + 3:7 * dt + 4],
                    in1=cv[:], op0=ALU.mult, op1=ALU.add)
                nc.scalar.activation(xc_sb[dt][:, lo:lo + LC], cv[:],
                                     ACTF.Silu,
                                     bias=chp_sb[:, 7 * dt + 4:7 * dt + 5],
                                     scale=1.0)

                # in_proj z rows -> silu -> zs (spill per chunk)
                ps_z = ps_pool.tile([128, LC], F32, name="ps_z", tag="ps_z")
                for kt in range(NKT):
                    nc.tensor.matmul(
                        ps_z[:],
                        lhsT=win_sb[kt][:, DH + dt * 128:DH + (dt + 1) * 128],
                        rhs=xt_sb[:, kt * LC:(kt + 1) * LC],
                        start=(kt == 0), stop=(kt == NKT - 1))
                zs_c = u_pool.tile([128, LC], BF16, name="zs_c", tag="zs_c")
                nc.scalar.activation(zs_c[:], ps_z[:], ACTF.Silu, scale=1.0)
                nc.sync.dma_start(sp_zs[dt * 128:(dt + 1) * 128, lo:lo + LC],
                                  zs_c[:])

            # x_dbl for this chunk
            ps96 = ps96_pool.tile([96, LC], F32, name="ps96", tag="ps96")
            for kt in range(NKT):
                nc.tensor.matmul(
                    ps96[:],
                    lhsT=wxp_sb[:, kt * 96:(kt + 1) * 96],
                    rhs=xc_sb[kt][:, lo:lo + LC],
                    start=(kt == 0), stop=(kt == NKT - 1))
            nc.scalar.copy(dt_sb[:, lo:lo + LC], ps96[0:64, :])
            nc.scalar.copy(bc_sb[:, lo:lo + LC], ps96[64:96, :])

            # spill xc/bc as chunks complete
            nc.sync.dma_start(sp_bc[:, lo:lo + LC], bc_sb[:, lo:lo + LC])
            for dt in range(NDT):
                nc.sync.dma_start(sp_xc[dt * 128:(dt + 1) * 128, lo:lo + LC],
                                  xc_sb[dt][:, lo:lo + LC])


def _phase2(nc, tc, sp_bc, sp_xc, sp_zs, sp_de, w_out,
            chp_sb, ident_sb, dt_sb, wdtp_sb, outp_a, outp_b):
    """dt-pair sweeps: each pair of d-tiles accumulates all 16 states in PSUM
    (2 dts x 4 quarter-banks = 8 banks), so the PSUM->SBUF tail runs once per
    dt.  B/C broadcast tiles rotate per state (bufs=2 prefetch)."""
    with (
        tc.tile_pool(name="b_wout", bufs=1) as wout_pool,
        tc.tile_pool(name="b_bc", bufs=2) as bc_pool,
        tc.tile_pool(name="b_yt", bufs=1) as yt_pool,
        tc.tile_pool(name="b_in", bufs=2) as in_pool,
        tc.tile_pool(name="b_a", bufs=3) as a_pool,
        tc.tile_pool(name="b_bh", bufs=3) as bh_pool,
        tc.tile_pool(name="b_tail", bufs=2) as tail_pool,
        tc.tile_pool(name="b_o", bufs=1) as o_pool,
        tc.tile_pool(name="b_ps", bufs=1, space="PSUM") as psy_pool,
    ):
        wout_sb = []
        for dt in range(NDT):
            t = wout_pool.tile([128, D_MODEL], BF16, name=f"wo{dt}",
                               tag=f"wo{dt}")
            nc.sync.dma_start(t[:], w_out[dt * 128:(dt + 1) * 128, :])
            wout_sb.append(t)

        yT = [yt_pool.tile([128, L], BF16, name=f"yT{dt}", tag=f"yT{dt}")
              for dt in range(NDT)]

        bm_idx = 0

        # softplus prologue: delta for all dts through 2 rotating psum banks,
        # spilled to DRAM so pairs can prefetch with no psum coupling
        for dt in range(NDT):
            for c in range(NLC):
                lo = c * LC
                psd = psy_pool.tile([128, LC], F32, name="psd",
                                    tag=f"ps{c % 2}0")
                nc.tensor.matmul(
                    psd[:],
                    lhsT=wdtp_sb[:, dt * 128:(dt + 1) * 128],
                    rhs=dt_sb[:, lo:lo + LC],
                    start=True, stop=True)
                u_t = tail_pool.tile([128, LC], BF16, name="u_t", tag="u_t")
                nc.scalar.activation(u_t[:], psd[:], ACTF.Exp,
                                     bias=chp_sb[:, 7 * dt + 5:7 * dt + 6],
                                     scale=1.0)
                de_c = tail_pool.tile([128, LC], BF16, name="de_c",
                                      tag="de_c")
                nc.scalar.activation(de_c[:], u_t[:], ACTF.Ln, bias=1.0,
                                     scale=1.0)
                nc.sync.dma_start(sp_de[dt * 128:(dt + 1) * 128, lo:lo + LC],
                                  de_c[:])

        def emit_pair_inputs(pair):
            dts = [2 * pair, 2 * pair + 1]
            tiles = {}
            for s, dt in enumerate(dts):
                dsl = slice(dt * 128, (dt + 1) * 128)
                xc = in_pool.tile([128, L], BF16, name="xc", tag=f"xc{s}")
                nc.sync.dma_start(xc[:], sp_xc[dsl, :])
                zs = in_pool.tile([128, L], BF16, name="zs", tag=f"zs{s}")
                nc.sync.dma_start(zs[:], sp_zs[dsl, :])
                de = in_pool.tile([128, L], BF16, name="de", tag=f"de{s}")
                nc.sync.dma_start(de[:], sp_de[dsl, :])
                du = in_pool.tile([128, L], BF16, name="du", tag=f"du{s}")
                nc.gpsimd.tensor_tensor(du[:], de[:], xc[:], op=ALU.mult)
                tiles[s] = (de, du, xc, zs)
            return tiles

        pending_out = [None]

        def emit_pending():
            if pending_out[0] is not None:
                half, mts, outp = pending_out[0]
                _out_proj_block(nc, psy_pool, o_pool, wout_sb, yT, half,
                                mts, outp)
                pending_out[0] = None

        tiles = emit_pair_inputs(0)
        for pair in range(NDT // 2):
            dts = [2 * pair, 2 * pair + 1]
            ps_y = {(s, q): psy_pool.tile([128, LC], F32, name=f"ps{s}{q}",
                                          tag=f"ps{s}{q}")
                    for s in range(2) for q in range(NLC)}

            for n in range(D_STATE):
                if n == 10:
                    emit_pending()
                Bn = bc_pool.tile([128, L], BF16, name=f"B{n}", tag="Bn")
                nc.sync.dma_start(Bn[:],
                                  sp_bc[n:n + 1, :].partition_broadcast(128))
                Cn = bc_pool.tile([128, L], BF16, name=f"C{n}", tag="Cn")
                nc.sync.dma_start(
                    Cn[:], sp_bc[16 + n:16 + n + 1, :].partition_broadcast(128))
                for s in range(2):
                    de, du, xc, zs = tiles[s]
                    a_t = a_pool.tile([128, L], F32, name=f"a{n}", tag="a")
                    nc.scalar.activation(a_t[:], de[:], ACTF.Exp,
                                         scale=-float(n + 1))
                    b_t = bh_pool.tile([128, L], BF16, name=f"b{n}", tag="b")
                    _bm_engine(nc, bm_idx, 0).tensor_tensor(
                        b_t[:], du[:], Bn[:], op=ALU.mult)
                    bm_idx += 1
                    h_t = bh_pool.tile([128, L], BF16, name=f"h{n}", tag="h")
                    nc.vector.tensor_tensor_scan(
                        h_t[:], a_t[:], b_t[:], 0.0,
                        op0=ALU.mult, op1=ALU.add)
                    m_t = bh_pool.tile([128, L], BF16, name=f"m{n}", tag="m")
                    _bm_engine(nc, bm_idx, 0).tensor_tensor(
                        m_t[:], h_t[:], Cn[:], op=ALU.mult)
                    bm_idx += 1
                    for q in range(NLC):
                        nc.tensor.matmul(
                            ps_y[(s, q)][:],
                            lhsT=ident_sb[:],
                            rhs=m_t[:, q * LC:(q + 1) * LC],
                            start=(n == 0),
                            stop=(n == D_STATE - 1))

            # tail once per dt: yT = (psum + xc*Dvec) * zs
            for s, dt in enumerate(dts):
                de, du, xc, zs = tiles[s]
                for q in range(NLC):
                    lo = q * LC
                    t_q = tail_pool.tile([128, LC], F32, name="t_q", tag="t_q")
                    nc.vector.scalar_tensor_tensor(
                        out=t_q[:], in0=xc[:, lo:lo + LC],
                        scalar=chp_sb[:, 7 * dt + 6:7 * dt + 7],
                        in1=ps_y[(s, q)][:], op0=ALU.mult, op1=ALU.add)
                    nc.gpsimd.tensor_tensor(yT[dt][:, lo:lo + LC], t_q[:],
                                            zs[:, lo:lo + LC],
                                            op=ALU.mult)

            if pair < NDT // 2 - 1:
                tiles = emit_pair_inputs(pair + 1)
            if pair == 1:
                pending_out[0] = (0, range(0, 4), outp_a)
            elif pair == 2:
                pending_out[0] = (0, range(4, 8), outp_a)
            elif pair == 3:
                emit_pending()
                _out_proj_block(nc, psy_pool, o_pool, wout_sb, yT, 1,
                                range(0, 8), outp_b)


def _out_proj_block(nc, psy_pool, o_pool, wout_sb, yT, half, mts, outp):
    dts = [half * 4 + i for i in range(4)]
    for mt in mts:
        o_t = o_pool.tile([128, L], BF16, name="o_t", tag="o_t")
        for c in range(NLC):
            ps = psy_pool.tile([128, LC], F32, name="ps_o",
                               tag=f"ps{c % 2}{c // 2}")
            for r, dt in enumerate(dts):
                nc.tensor.matmul(
                    ps[:],
                    lhsT=wout_sb[dt][:, mt * 128:(mt + 1) * 128],
                    rhs=yT[dt][:, c * LC:(c + 1) * LC],
                    start=(r == 0), stop=(r == 3))
            nc.scalar.copy(o_t[:, c * LC:(c + 1) * LC], ps[:])
        nc.sync.dma_start(outp[mt * 128:(mt + 1) * 128, :], o_t[:])


def make_in_maps(inputs):
    import ml_dtypes
    bf16 = ml_dtypes.bfloat16
    x = np.asarray(inputs["x"], np.float32)
    names = ["in_w", "conv_w", "conv_b", "xp_w", "dtp_w", "dtp_b",
             "A_log", "Dvec", "out_w"]
    params = {d: [np.asarray(inputs[k + str(d + 1)], np.float32) for k in names]
              for d in range(2)}
    # the device program hardcodes A_n = -(n+1); verify
    expA = np.log(np.arange(1, D_STATE + 1, dtype=np.float32))
    for d in range(2):
        A_log = params[d][6]
        assert np.allclose(A_log, np.broadcast_to(expA, A_log.shape),
                           atol=1e-6), \
            "A_log does not match the expected log(arange(1,17)) pattern"

    ident = np.eye(128, dtype=np.float32)

    in_maps, metas = [], []
    for core in range(8):
        b = core & 1
        dire = (core >> 1) & 1
        half = (core >> 2) & 1
        in_w, conv_w, conv_b, xp_w, dtp_w, dtp_b, A_log, Dp, out_w = params[dire]
        sl = slice(half * DH, (half + 1) * DH)
        xb = x[b] if dire == 0 else x[b, ::-1]
        chpm = np.zeros((128, NDT * 7), np.float32)
        for dt in range(NDT):
            ch = slice(half * DH + dt * 128, half * DH + (dt + 1) * 128)
            for k in range(4):
                chpm[:, 7 * dt + k] = conv_w[ch, 0, k]
            chpm[:, 7 * dt + 4] = conv_b[ch]
            chpm[:, 7 * dt + 5] = dtp_b[ch]
            chpm[:, 7 * dt + 6] = Dp[ch]
        in_maps.append({
            "xT": np.ascontiguousarray(xb.T).astype(bf16),
            "w_in": np.ascontiguousarray(
                np.concatenate([in_w[sl], in_w[D_INNER + half * DH:
                                               D_INNER + (half + 1) * DH]]).T
            ).astype(bf16),
            "w_xp": np.ascontiguousarray(xp_w[:, sl].T).astype(bf16),
            "w_dtp": np.ascontiguousarray(dtp_w[sl].T).astype(bf16),
            "w_out": np.ascontiguousarray(out_w[:, sl].T).astype(bf16),
            "ident": ident.astype(bf16),
            "chp": chpm.astype(np.float32),
        })
        metas.append(b)
    return in_maps, metas


_PROGRAM_CACHE = {}


def kernel(**inputs):
    global LAST_EXEC_NS
    import os
    from concourse.bass_utils import run_bass_kernel_spmd

    if "nc" not in _PROGRAM_CACHE:
        _PROGRAM_CACHE["nc"] = build_program()
    nc = _PROGRAM_CACHE["nc"]

    in_maps, metas = make_in_maps(inputs)
    trace = os.environ.get("BIMAMBA_TRACE", "0") == "1"
    res = run_bass_kernel_spmd(nc, in_maps, list(range(8)), trace=trace)
    LAST_EXEC_NS = res.exec_time_ns
    out = np.zeros((B, L, D_MODEL), np.float32)
    for core in range(8):
        out[metas[core]] += np.asarray(res.results[core]["outp_a"],
                                       np.float32).T
        out[metas[core]] += np.asarray(res.results[core]["outp_b"],
                                       np.float32).T
    return out


# revision 45
# speedup vs baseline: 1.3165x; 1.0578x over previous
"""BiMamba Trainium2 kernel (v3).

8-core sharding: core = (batch b) x (direction) x (d_inner half).  Each core
runs one Mamba branch over DH=1024 channels for one batch element; host sums
the 4 partials per batch element.

Structure:
  - Phase 1: in_proj (xi and z rows), depthwise conv (Pool/DVE MACs), silu,
    x_dbl on PE; xc/zs/B/C spill per chunk to DRAM (no phase-boundary DMA
    wall).
  - Phase 2 prologue: dt_proj + softplus for all d-tiles through 2 rotating
    PSUM banks, spilled to DRAM so the scan pairs prefetch delta via DMA with
    no PSUM coupling.
  - Phase 2: d-tile PAIRS, each sweeping all 16 SSM states; the state sum
    accumulates in PSUM f32 via identity matmuls (2 dts x 4 quarter-banks =
    all 8 banks), so the PSUM->SBUF tail runs once per d-tile.  B/C broadcast
    tiles rotate per state with DMA prefetch.  a = exp(-(n+1)*delta) on ACT,
    b = du*B and m = h*C split DVE/Pool (2/9 on DVE), scans on DVE (the hard
    floor: 128 x 2.2us).  out_proj sub-blocks are interleaved into later
    pairs' state sweeps.
  - Compiler constraints honored: GPSIMD never touches PSUM and never runs
    scalar_tensor_tensor; scans are DVE-only.
  - bf16 everywhere except a-tiles and PSUM; outputs bf16, summed on host.

The exp scale -(n+1) relies on A_log = log(arange(1, 17)) broadcast over
channels, which setup_inputs() guarantees; kernel() asserts it.
"""

import sys

for _p in ("/opt/trn_rl_repo",):
    if _p not in sys.path:
        sys.path.insert(0, _p)

import numpy as np

import concourse.bass as bass
import concourse.bacc as bacc
import concourse.mybir as mybir
import concourse.tile as tile

# Model dims (hardcoded per contest contract)
D_MODEL = 1024
D_STATE = 16
D_INNER = 2048
DT_RANK = 64
B, L = 2, 2048
DH = D_INNER // 2          # 1024 channels per core
NDT = DH // 128            # 8 d-tiles per core
NKT = D_MODEL // 128       # 8 k-tiles for in_proj contraction

F32 = mybir.dt.float32
BF16 = mybir.dt.bfloat16
ALU = mybir.AluOpType
ACTF = mybir.ActivationFunctionType

LC = 512                   # phase-1 L-chunk (psum bank width in f32)
NLC = L // LC
NG = 8                     # n-group size in phase 2 (B/C tiles resident)

LAST_EXEC_NS = None


def _bm_engine(nc, idx, g):
    """Engine for the b/m broadcast multiplies: DVE share tuned per group."""
    return nc.vector if (idx % 9) < 2 else nc.gpsimd


def build_program():
    nc = bacc.Bacc("TRN2", target_bir_lowering=False, debug=False,
                   num_devices=8)

    xT = nc.dram_tensor("xT", [D_MODEL, L], BF16, kind="ExternalInput")
    w_in = nc.dram_tensor("w_in", [D_MODEL, 2 * DH], BF16, kind="ExternalInput")
    w_xp = nc.dram_tensor("w_xp", [DH, 96], BF16, kind="ExternalInput")
    w_dtp = nc.dram_tensor("w_dtp", [DT_RANK, DH], BF16, kind="ExternalInput")
    w_out = nc.dram_tensor("w_out", [DH, D_MODEL], BF16, kind="ExternalInput")
    ident = nc.dram_tensor("ident", [128, 128], BF16, kind="ExternalInput")
    # per-channel params per dt: conv taps 0-3, conv_b, dtp_b, Dvec
    chp = nc.dram_tensor("chp", [128, NDT * 7], F32, kind="ExternalInput")
    outp_a = nc.dram_tensor("outp_a", [D_MODEL, L], BF16, kind="ExternalOutput")
    outp_b = nc.dram_tensor("outp_b", [D_MODEL, L], BF16, kind="ExternalOutput")

    sp_bc = nc.dram_tensor("sp_bc", [32, L], BF16)
    sp_xc = nc.dram_tensor("sp_xc", [DH, L], BF16)
    sp_zs = nc.dram_tensor("sp_zs", [DH, L], BF16)
    sp_de = nc.dram_tensor("sp_de", [DH, L], BF16)

    with tile.TileContext(nc) as tc:
        with (
            tc.tile_pool(name="const", bufs=1) as const_pool,
        ):
            ident_sb = const_pool.tile([128, 128], BF16, name="ident",
                                       tag="ident")
            nc.sync.dma_start(ident_sb[:], ident[:])
            chp_sb = const_pool.tile([128, NDT * 7], F32, name="chp", tag="chp")
            nc.sync.dma_start(chp_sb[:], chp[:])
            dt_sb = const_pool.tile([DT_RANK, L], BF16, name="dt_sb",
                                    tag="dt_sb")
            wdtp_sb = const_pool.tile([DT_RANK, DH], BF16, name="wdtp",
                                      tag="wdtp")
            nc.sync.dma_start(wdtp_sb[:], w_dtp[:])

            _phase1(nc, tc, xT, w_in, w_xp, chp_sb, dt_sb, wdtp_sb,
                    sp_bc, sp_xc, sp_zs, sp_de)
            _phase2(nc, tc, sp_bc, sp_xc, sp_zs, sp_de, w_out,
                    chp_sb, ident_sb, dt_sb, wdtp_sb, outp_a, outp_b)
    nc.finalize()
    return nc


def _phase1(nc, tc, xT, w_in, w_xp, chp_sb, dt_sb, wdtp_sb,
            sp_bc, sp_xc, sp_zs, sp_de):
    # in_proj + conv(Pool) + silu; x_dbl / dt_proj / softplus / du
    # interleaved per chunk so ACT and Pool trail the PE in_proj stream.
    with (
        tc.tile_pool(name="a_big", bufs=1) as big_pool,      # xc, zs, de, du
        tc.tile_pool(name="a_small", bufs=1) as small_pool,  # dt_sb, bc_sb
        tc.tile_pool(name="a_win", bufs=1) as win_pool,
        tc.tile_pool(name="a_xt", bufs=2) as xt_pool,
        tc.tile_pool(name="a_xi", bufs=2) as xi_pool,
        tc.tile_pool(name="a_u", bufs=2) as u_pool,
        tc.tile_pool(name="a_ps", bufs=2, space="PSUM") as ps_pool,
        tc.tile_pool(name="a_ps96", bufs=2, space="PSUM") as ps96_pool,
    ):
        xc_sb = [big_pool.tile([128, L], BF16, name=f"xc{dt}", tag=f"xc{dt}")
                 for dt in range(NDT)]
        bc_sb = small_pool.tile([32, L], BF16, name="bc_sb", tag="bc_sb")

        win_sb = []
        for kt in range(NKT):
            t = win_pool.tile([128, 2 * DH], BF16, name=f"win{kt}",
                              tag=f"win{kt}")
            nc.sync.dma_start(t[:], w_in[kt * 128:(kt + 1) * 128, :])
            win_sb.append(t)
        wxp_sb = win_pool.tile([128, NKT * 96], BF16, name="wxp", tag="wxp")
        nc.sync.dma_start(
            wxp_sb[:].rearrange("p (a l) -> p a l", a=NKT),
            w_xp[:].rearrange("(a p) l -> p a l", p=128))
        hist = [None] * NDT
        for c in range(NLC):
            lo = c * LC
            xt_sb = xt_pool.tile([128, NKT * LC], BF16, name="xt", tag="xt")
            nc.sync.dma_start(
                xt_sb[:].rearrange("p (a l) -> p a l", a=NKT),
                xT[:, lo:lo + LC].rearrange("(a p) l -> p a l", p=128))

            for dt in range(NDT):
                # in_proj xi rows
                ps = ps_pool.tile([128, LC], F32, name="ps_xi", tag="ps_xi")
                for kt in range(NKT):
                    nc.tensor.matmul(
                        ps[:],
                        lhsT=win_sb[kt][:, dt * 128:(dt + 1) * 128],
                        rhs=xt_sb[:, kt * LC:(kt + 1) * LC],
                        start=(kt == 0), stop=(kt == NKT - 1))
                xi = xi_pool.tile([128, LC + 3], BF16, name="xi", tag="xi",
                                  bufs=3)
                if c == 0:
                    nc.vector.memset(xi[:, 0:3], 0.0)
                else:
                    nc.vector.tensor_copy(xi[:, 0:3], hist[dt][:])
                nc.vector.tensor_copy(xi[:, 3:LC + 3], ps[:])
                if c < NLC - 1:
                    h_t = xi_pool.tile([128, 3], BF16, name="hist",
                                       tag=f"hist{dt}", bufs=2)
                    nc.vector.tensor_copy(h_t[:], xi[:, LC:LC + 3])
                    hist[dt] = h_t

                # conv: 4 causal taps on Pool (f32 accumulate)
                cv = xi_pool.tile([128, LC], F32, name="cv", tag="cv", bufs=2)
                nc.gpsimd.tensor_scalar(cv[:], xi[:, 0:LC],
                                        chp_sb[:, 7 * dt:7 * dt + 1], None,
                                        op0=ALU.mult)
                nc.vector.scalar_tensor_tensor(
                    out=cv[:], in0=xi[:, 1:1 + LC],
                    scalar=chp_sb[:, 7 * dt + 1:7 * dt + 2],
                    in1=cv[:], op0=ALU.mult, op1=ALU.add)
                cv2 = xi_pool.tile([128, LC], F32, name="cv2", tag="cv2",
                                   bufs=2)
                nc.gpsimd.tensor_scalar(cv2[:], xi[:, 2:2 + LC],
                                        chp_sb[:, 7 * dt + 2:7 * dt + 3],
                                        None, op0=ALU.mult)
                nc.gpsimd.tensor_tensor(cv[:], cv[:], cv2[:], op=ALU.add)
                nc.vector.scalar_tensor_tensor(
                    out=cv[:], in0=xi[:, 3:3 + LC],
                    scalar=chp_sb[:, 7 * dt + 3:7 * dt + 4],
                    in1=cv[:], op0=ALU.mult, op1=ALU.add)
                nc.scalar.activation(xc_sb[dt][:, lo:lo + LC], cv[:],
                                     ACTF.Silu,
                                     bias=chp_sb[:, 7 * dt + 4:7 * dt + 5],
                                     scale=1.0)

                # in_proj z rows -> silu -> zs (spill per chunk)
                ps_z = ps_pool.tile([128, LC], F32, name="ps_z", tag="ps_z")
                for kt in range(NKT):
                    nc.tensor.matmul(
                        ps_z[:],
                        lhsT=win_sb[kt][:, DH + dt * 128:DH + (dt + 1) * 128],
                        rhs=xt_sb[:, kt * LC:(kt + 1) * LC],
                        start=(kt == 0), stop=(kt == NKT - 1))
                zs_c = u_pool.tile([128, LC], BF16, name="zs_c", tag="zs_c")
                nc.scalar.activation(zs_c[:], ps_z[:], ACTF.Silu, scale=1.0)
                nc.sync.dma_start(sp_zs[dt * 128:(dt + 1) * 128, lo:lo + LC],
                                  zs_c[:])

            # x_dbl for this chunk
            ps96 = ps96_pool.tile([96, LC], F32, name="ps96", tag="ps96")
            for kt in range(NKT):
                nc.tensor.matmul(
                    ps96[:],
                    lhsT=wxp_sb[:, kt * 96:(kt + 1) * 96],
                    rhs=xc_sb[kt][:, lo:lo + LC],
                    start=(kt == 0), stop=(kt == NKT - 1))
            nc.vector.tensor_copy(dt_sb[:, lo:lo + LC], ps96[0:64, :])
            nc.vector.tensor_copy(bc_sb[:, lo:lo + LC], ps96[64:96, :])

            # dt_proj + softplus for this chunk (hides in per-chunk ACT slack)
            for dt in range(NDT):
                psd = ps_pool.tile([128, LC], F32, name="psd", tag="psd")
                nc.tensor.matmul(
                    psd[:],
                    lhsT=wdtp_sb[:, dt * 128:(dt + 1) * 128],
                    rhs=dt_sb[:, lo:lo + LC],
                    start=True, stop=True)
                u_t = u_pool.tile([128, LC], BF16, name="u_t", tag="u_t")
                nc.scalar.activation(u_t[:], psd[:], ACTF.Exp,
                                     bias=chp_sb[:, 7 * dt + 5:7 * dt + 6],
                                     scale=1.0)
                de_c = u_pool.tile([128, LC], BF16, name="de_c", tag="de_c")
                nc.scalar.activation(de_c[:], u_t[:], ACTF.Ln, bias=1.0,
                                     scale=1.0)
                nc.sync.dma_start(sp_de[dt * 128:(dt + 1) * 128, lo:lo + LC],
                                  de_c[:])

            # spill xc/bc as chunks complete
            nc.sync.dma_start(sp_bc[:, lo:lo + LC], bc_sb[:, lo:lo + LC])
            for dt in range(NDT):
                nc.sync.dma_start(sp_xc[dt * 128:(dt + 1) * 128, lo:lo + LC],
                                  xc_sb[dt][:, lo:lo + LC])


def _phase2(nc, tc, sp_bc, sp_xc, sp_zs, sp_de, w_out,
            chp_sb, ident_sb, dt_sb, wdtp_sb, outp_a, outp_b):
    """dt-pair sweeps: each pair of d-tiles accumulates all 16 states in PSUM
    (2 dts x 4 quarter-banks = 8 banks), so the PSUM->SBUF tail runs once per
    dt.  B/C broadcast tiles rotate per state (bufs=2 prefetch)."""
    with (
        tc.tile_pool(name="b_wout", bufs=1) as wout_pool,
        tc.tile_pool(name="b_bc", bufs=2) as bc_pool,
        tc.tile_pool(name="b_yt", bufs=1) as yt_pool,
        tc.tile_pool(name="b_in", bufs=2) as in_pool,
        tc.tile_pool(name="b_a", bufs=3) as a_pool,
        tc.tile_pool(name="b_bh", bufs=3) as bh_pool,
        tc.tile_pool(name="b_tail", bufs=2) as tail_pool,
        tc.tile_pool(name="b_o", bufs=1) as o_pool,
        tc.tile_pool(name="b_ps", bufs=1, space="PSUM") as psy_pool,
    ):
        wout_sb = []
        for dt in range(NDT):
            t = wout_pool.tile([128, D_MODEL], BF16, name=f"wo{dt}",
                               tag=f"wo{dt}")
            nc.sync.dma_start(t[:], w_out[dt * 128:(dt + 1) * 128, :])
            wout_sb.append(t)

        yT = [yt_pool.tile([128, L], BF16, name=f"yT{dt}", tag=f"yT{dt}")
              for dt in range(NDT)]

        bm_idx = 0

        def emit_pair_inputs(pair):
            dts = [2 * pair, 2 * pair + 1]
            tiles = {}
            for s, dt in enumerate(dts):
                dsl = slice(dt * 128, (dt + 1) * 128)
                xc = in_pool.tile([128, L], BF16, name="xc", tag=f"xc{s}")
                nc.sync.dma_start(xc[:], sp_xc[dsl, :])
                zs = in_pool.tile([128, L], BF16, name="zs", tag=f"zs{s}")
                nc.sync.dma_start(zs[:], sp_zs[dsl, :])
                de = in_pool.tile([128, L], BF16, name="de", tag=f"de{s}")
                nc.sync.dma_start(de[:], sp_de[dsl, :])
                du = in_pool.tile([128, L], BF16, name="du", tag=f"du{s}")
                nc.gpsimd.tensor_tensor(du[:], de[:], xc[:], op=ALU.mult)
                tiles[s] = (de, du, xc, zs)
            return tiles

        pending_out = [None]

        def emit_pending():
            if pending_out[0] is not None:
                half, mts, outp = pending_out[0]
                _out_proj_block(nc, psy_pool, o_pool, wout_sb, yT, half,
                                mts, outp)
                pending_out[0] = None

        tiles = emit_pair_inputs(0)
        for pair in range(NDT // 2):
            dts = [2 * pair, 2 * pair + 1]
            ps_y = {(s, q): psy_pool.tile([128, LC], F32, name=f"ps{s}{q}",
                                          tag=f"ps{s}{q}")
                    for s in range(2) for q in range(NLC)}

            for n in range(D_STATE):
                if n == 10:
                    emit_pending()
                Bn = bc_pool.tile([128, L], BF16, name=f"B{n}", tag="Bn")
                nc.sync.dma_start(Bn[:],
                                  sp_bc[n:n + 1, :].partition_broadcast(128))
                Cn = bc_pool.tile([128, L], BF16, name=f"C{n}", tag="Cn")
                nc.sync.dma_start(
                    Cn[:], sp_bc[16 + n:16 + n + 1, :].partition_broadcast(128))
                for s in range(2):
                    de, du, xc, zs = tiles[s]
                    a_t = a_pool.tile([128, L], F32, name=f"a{n}", tag="a")
                    nc.scalar.activation(a_t[:], de[:], ACTF.Exp,
                                         scale=-float(n + 1))
                    b_t = bh_pool.tile([128, L], BF16, name=f"b{n}", tag="b")
                    _bm_engine(nc, bm_idx, 0).tensor_tensor(
                        b_t[:], du[:], Bn[:], op=ALU.mult)
                    bm_idx += 1
                    h_t = bh_pool.tile([128, L], BF16, name=f"h{n}", tag="h")
                    nc.vector.tensor_tensor_scan(
                        h_t[:], a_t[:], b_t[:], 0.0,
                        op0=ALU.mult, op1=ALU.add)
                    m_t = bh_pool.tile([128, L], BF16, name=f"m{n}", tag="m")
                    _bm_engine(nc, bm_idx, 0).tensor_tensor(
                        m_t[:], h_t[:], Cn[:], op=ALU.mult)
                    bm_idx += 1
                    for q in range(NLC):
                        nc.tensor.matmul(
                            ps_y[(s, q)][:],
                            lhsT=ident_sb[:],
                            rhs=m_t[:, q * LC:(q + 1) * LC],
                            start=(n == 0),
                            stop=(n == D_STATE - 1))

            # tail once per dt: yT = (psum + xc*Dvec) * zs
            for s, dt in enumerate(dts):
                de, du, xc, zs = tiles[s]
                for q in range(NLC):
                    lo = q * LC
                    t_q = tail_pool.tile([128, LC], F32, name="t_q", tag="t_q")
                    nc.vector.scalar_tensor_tensor(
                        out=t_q[:], in0=xc[:, lo:lo + LC],
                        scalar=chp_sb[:, 7 * dt + 6:7 * dt + 7],
                        in1=ps_y[(s, q)][:], op0=ALU.mult, op1=ALU.add)
                    nc.gpsimd.tensor_tensor(yT[dt][:, lo:lo + LC], t_q[:],
                                            zs[:, lo:lo + LC],
                                            op=ALU.mult)

            if pair < NDT // 2 - 1:
                tiles = emit_pair_inputs(pair + 1)
            if pair == 1:
                pending_out[0] = (0, range(0, 4), outp_a)
            elif pair == 2:
                pending_out[0] = (0, range(4, 8), outp_a)
            elif pair == 3:
                emit_pending()
                _out_proj_block(nc, psy_pool, o_pool, wout_sb, yT, 1,
                                range(0, 8), outp_b)


def _out_proj_block(nc, psy_pool, o_pool, wout_sb, yT, half, mts, outp):
    dts = [half * 4 + i for i in range(4)]
    for mt in mts:
        o_t = o_pool.tile([128, L], BF16, name="o_t", tag="o_t")
        for c in range(NLC):
            ps = psy_pool.tile([128, LC], F32, name="ps_o",
                               tag=f"ps{c % 2}{c // 2}")
            for r, dt in enumerate(dts):
                nc.tensor.matmul(
                    ps[:],
                    lhsT=wout_sb[dt][:, mt * 128:(mt + 1) * 128],
                    rhs=yT[dt][:, c * LC:(c + 1) * LC],
                    start=(r == 0), stop=(r == 3))
            nc.scalar.copy(o_t[:, c * LC:(c + 1) * LC], ps[:])
        nc.sync.dma_start(outp[mt * 128:(mt + 1) * 128, :], o_t[:])


def make_in_maps(inputs):
    import ml_dtypes
    bf16 = ml_dtypes.bfloat16
    x = np.asarray(inputs["x"], np.float32)
    names = ["in_w", "conv_w", "conv_b", "xp_w", "dtp_w", "dtp_b",
             "A_log", "Dvec", "out_w"]
    params = {d: [np.asarray(inputs[k + str(d + 1)], np.float32) for k in names]
              for d in range(2)}
    # the device program hardcodes A_n = -(n+1); verify
    expA = np.log(np.arange(1, D_STATE + 1, dtype=np.float32))
    for d in range(2):
        A_log = params[d][6]
        assert np.allclose(A_log, np.broadcast_to(expA, A_log.shape),
                           atol=1e-6), \
            "A_log does not match the expected log(arange(1,17)) pattern"

    ident = np.eye(128, dtype=np.float32)

    in_maps, metas = [], []
    for core in range(8):
        b = core & 1
        dire = (core >> 1) & 1
        half = (core >> 2) & 1
        in_w, conv_w, conv_b, xp_w, dtp_w, dtp_b, A_log, Dp, out_w = params[dire]
        sl = slice(half * DH, (half + 1) * DH)
        xb = x[b] if dire == 0 else x[b, ::-1]
        chpm = np.zeros((128, NDT * 7), np.float32)
        for dt in range(NDT):
            ch = slice(half * DH + dt * 128, half * DH + (dt + 1) * 128)
            for k in range(4):
                chpm[:, 7 * dt + k] = conv_w[ch, 0, k]
            chpm[:, 7 * dt + 4] = conv_b[ch]
            chpm[:, 7 * dt + 5] = dtp_b[ch]
            chpm[:, 7 * dt + 6] = Dp[ch]
        in_maps.append({
            "xT": np.ascontiguousarray(xb.T).astype(bf16),
            "w_in": np.ascontiguousarray(
                np.concatenate([in_w[sl], in_w[D_INNER + half * DH:
                                               D_INNER + (half + 1) * DH]]).T
            ).astype(bf16),
            "w_xp": np.ascontiguousarray(xp_w[:, sl].T).astype(bf16),
            "w_dtp": np.ascontiguousarray(dtp_w[sl].T).astype(bf16),
            "w_out": np.ascontiguousarray(out_w[:, sl].T).astype(bf16),
            "ident": ident.astype(bf16),
            "chp": chpm.astype(np.float32),
        })
        metas.append(b)
    return in_maps, metas


_PROGRAM_CACHE = {}


def kernel(**inputs):
    global LAST_EXEC_NS
    import os
    from concourse.bass_utils import run_bass_kernel_spmd

    if "nc" not in _PROGRAM_CACHE:
        _PROGRAM_CACHE["nc"] = build_program()
    nc = _PROGRAM_CACHE["nc"]

    in_maps, metas = make_in_maps(inputs)
    trace = os.environ.get("BIMAMBA_TRACE", "0") == "1"
    res = run_bass_kernel_spmd(nc, in_maps, list(range(8)), trace=trace)
    LAST_EXEC_NS = res.exec_time_ns
    out = np.zeros((B, L, D_MODEL), np.float32)
    for core in range(8):
        out[metas[core]] += np.asarray(res.results[core]["outp_a"],
                                       np.float32).T
        out[metas[core]] += np.asarray(res.results[core]["outp_b"],
                                       np.float32).T
    return out
